# revision 16
# baseline (speedup 1.0000x reference)
"""Binarized 3x3 conv (NCHW, VALID, stride 1) on 8 Trainium2 NeuronCores.

Reference: out = conv2d(X, sign(W)) with X [32,256,56,56] f32, W [256,256,3,3]
f32 (OIHW), out [32,256,54,54].

Sharding (per the data-parallel hint): each of the 8 cores gets 4 images of
the batch; the (tiny) weight is replicated.  No collectives.  The host only
re-lays-out W to [kh*kw, ci, co] (pure transpose, no arithmetic); sign() runs
on device.

kernel() uses build_kernel: 1D Winograd F(2,3) along HEIGHT with flat
contiguous matmul windows.  Binarized +-1 weights make the Winograd weight
transform EXACT in bf16 ({+-1/2, +-1, +-3/2}); the only numeric loss vs the
f32 reference is bf16 rounding of the input / its transform (~3e-3 max-rel
on the output, tolerance is 2e-2).

Why this shape is fast (measured via the test.py For_i rep fit, 1 core):
  - v0 direct conv, 9 shifted bf16 matmuls:            ~215 us/core
  - v1 F(2,3) along width (strided comp slices):       ~153 us/core
  - v2 F(2,3) along height, flat rhs  (this kernel):   ~65 us/core
The jump comes from the matmul moving operand: with a fully CONTIGUOUS
bf16 rhs (flat 504-element windows, 2 garbage columns per 56-wide row
discarded at detransform) the PE streams ~2 columns/cycle, i.e. 576
matmuls x [K=128, N=504] ~ 62 us.  Strided rhs ([9 rows, 54 of 56])
breaks that (~99 us/core), and the width-direction Winograd additionally
forces strided (stride-2) DVE transforms at 1 elem/lane/cycle.

Per-core engine budget for v2: PE ~62 us (the bound); DVE ~62 us (height
transforms are unit-stride bf16 tensor_tensor at 2x packed mode + f32
detransform); ACT: f32->bf16 input casts, one PSUM staging copy per group,
output-DMA ring; SP ring: chunked input DMA.  NTFF profiling is unavailable
under this axon image, so HW exec time is the wall(R) = a + E*R fit over
device-side For_i reps; wall-clock jitter of the axon tunnel makes single
fits noisy -- test.py takes min-of-12 per R.
"""

import numpy as np

_N, _C, _H, _W = 32, 256, 56, 56
_CO, _KH, _KW = 256, 3, 3
_HO, _WO = 54, 54
_NCORES = 8
_NPC = _N // _NCORES  # images per core

_R = 9             # output rows per PSUM group
_G = _HO // _R     # 6 row groups
_NF = _R * _W      # 504 = matmul free size
_PAD = 8           # bf16 image pad so the last rhs slice stays in bounds


def build_conv_bass(
    npc=_NPC,
    reps=1,
    free2d=True,
    w_on_act=True,
    cast_chunks=6,
    hw_loop=0,
    wswap=False,
    xb_bufs=4,
    ob_bufs=3,
):
    import contextlib

    import concourse.mybir as mybir
    import concourse.tile as tile
    from concourse import bacc

    fp32 = mybir.dt.float32
    bf16 = mybir.dt.bfloat16

    nc = bacc.Bacc("TRN2", target_bir_lowering=False, debug=False)

    x_in = nc.dram_tensor("x", [npc, _C, _H, _W], fp32, kind="ExternalInput")
    w_in = nc.dram_tensor("w", [_KH * _KW, _C, _CO], fp32, kind="ExternalInput")
    out = nc.dram_tensor("out", [npc, _CO, _HO, _WO], fp32, kind="ExternalOutput")

    n_ci = _C // 128   # 2
    n_co = _CO // 128  # 2
    nk = _KH * _KW     # 9

    with tile.TileContext(nc) as tc:
        with (
            tc.tile_pool(name="wstage", bufs=2) as wstage_pool,
            tc.tile_pool(name="wb", bufs=n_ci) as wb_pool,
            tc.tile_pool(name="xf", bufs=2) as xf_pool,
            tc.tile_pool(name="xb", bufs=xb_bufs) as xb_pool,
            tc.tile_pool(name="ob", bufs=ob_bufs) as ob_pool,
            tc.tile_pool(name="ps", bufs=8, space="PSUM") as ps_pool,
        ):
            # ---- weight prep: one DMA + one binarize per ci tile.
            # wb[ci_t][:, khw, co] = 0.5*sign(W[co, ci, khw]) in bf16 (exact)
            wb = {}
            w_dma_eng = nc.scalar if w_on_act else nc.sync
            for ci_t in range(n_ci):
                stage = wstage_pool.tile([128, nk, _CO], fp32)
                w_dma_eng.dma_start(
                    stage[:],
                    w_in[:, ci_t * 128 : (ci_t + 1) * 128, :].rearrange(
                        "k c o -> c k o"
                    ),
                )
                wt = wb_pool.tile([128, nk, _CO], bf16)
                # (w >= 0) -> {1,0}; minus 0.5 -> {+0.5,-0.5} == sign(w)/2
                nc.vector.tensor_scalar(
                    wt[:], stage[:], 0.0, 0.5,
                    mybir.AluOpType.is_ge, mybir.AluOpType.subtract,
                )
                wb[ci_t] = wt

            # ---- main loop over images
            # hw_loop>0: wrap the whole image loop in a device-side For_i
            # (loop var unused; used only to scale exec time for benchmarking)
            loop_cm = (
                tc.For_i(
                    0,
                    hw_loop,
                    1,
                    hint_engines=(
                        mybir.EngineType.PE,
                        mybir.EngineType.Activation,
                        mybir.EngineType.DVE,
                        mybir.EngineType.SP,
                    ),
                )
                if hw_loop > 0
                else contextlib.nullcontext()
            )
            with loop_cm:
                for rep in range(reps):
                    for n in range(npc):
                        xb = {}
                        for ci_t in range(n_ci):
                            xf = xf_pool.tile([128, _H * _W], fp32)
                            nc.sync.dma_start(
                                xf[:], x_in[n, ci_t * 128 : (ci_t + 1) * 128, :, :]
                            )
                            xt = xb_pool.tile([128, _H * _W + _PAD], bf16)
                            nc.vector.memset(xt[:, _H * _W :], 0.0)
                            # chunked cast so the first matmuls start sooner
                            hw = _H * _W
                            step = -(-hw // cast_chunks)
                            for s in range(0, hw, step):
                                e = min(s + step, hw)
                                nc.vector.tensor_copy(xt[:, s:e], xf[:, s:e])
                            xb[ci_t] = xt
    
                        for co_t in range(n_co):
                            pshape = [128, _R, _WO] if free2d else [128, _R, _W]
                            psts = [
                                ps_pool.tile(pshape, fp32, name="pst", tag="pst")
                                for _ in range(_G)
                            ]
                            taps = [
                                (ci_t, kh, kw)
                                for ci_t in range(n_ci)
                                for kh in range(_KH)
                                for kw in range(_KW)
                            ]
                            # weight-stationary (w outer, g inner) unless wswap
                            mm_iter = (
                                [(t, g) for g in range(_G) for t in taps]
                                if wswap
                                else [(t, g) for t in taps for g in range(_G)]
                            )
                            for (ci_t, kh, kw), g in mm_iter:
                                w_ap = wb[ci_t][
                                    :, kh * _KW + kw,
                                    co_t * 128 : (co_t + 1) * 128,
                                ]
                                first = ci_t == 0 and kh == 0 and kw == 0
                                last = (
                                    ci_t == n_ci - 1
                                    and kh == _KH - 1
                                    and kw == _KW - 1
                                )
                                base = (g * _R + kh) * _W + kw
                                if free2d:
                                    rhs = xb[ci_t][
                                        :, base : base + _NF
                                    ].rearrange("p (r w) -> p r w", r=_R)[
                                        :, :, 0:_WO
                                    ]
                                else:
                                    rhs = xb[ci_t][:, base : base + _NF]
                                nc.tensor.matmul(
                                    psts[g][:, :, :],
                                    w_ap,
                                    rhs,
                                    start=first,
                                    stop=last,
                                )
                            ob = ob_pool.tile([128, _HO, _WO], fp32)
                            for g in range(_G):
                                # x2 undoes the +-0.5 weight encoding (exact)
                                nc.scalar.mul(
                                    ob[:, g * _R : (g + 1) * _R, :],
                                    psts[g][:, :, 0:_WO] if not free2d else psts[g][:],
                                    2.0,
                                )
                            nc.scalar.dma_start(
                                out[n, co_t * 128 : (co_t + 1) * 128, :, :], ob[:]
                            )

    nc.compile()
    return nc


def build_conv_bass_wino(
    npc=_NPC,
    reps=1,
    hw_loop=0,
    rg=18,
    xw_bufs=4,
    ob_bufs=2,
    dt_bufs=8,
    tr_chunks=2,
):
    """1D Winograd F(2,3) along width: 1.5x fewer PE cycles than direct.

    For output pair (y0,y1) at width 2j with taps (w0,w1,w2) per (ci,kh):
      c1=d0-d2, c2=d1+d2, c3=d2-d1, c4=d1-d3   (dt = x[2j+t])
      m1=c1*w0, m2=c2*(w0+w1+w2)/2, m3=c3*(w0-w1+w2)/2, m4=c4*w2
      y0 = m1+m2+m3,  y1 = m2-m3-m4
    Signs are binarized first, so transformed weights are in
    {+-1, +-1/2, +-3/2} — exact in bf16.  Input comps are computed on DVE
    straight from the f32 image (cast fused).  The four m accumulators are
    one PSUM quad per row-group; detransform is 4 DVE adds per quad.
    """
    import contextlib

    import concourse.mybir as mybir
    import concourse.tile as tile
    from concourse import bacc

    fp32 = mybir.dt.float32
    bf16 = mybir.dt.bfloat16

    nc = bacc.Bacc("TRN2", target_bir_lowering=False, debug=False)

    x_in = nc.dram_tensor("x", [npc, _C, _H, _W], fp32, kind="ExternalInput")
    w_in = nc.dram_tensor("w", [_KH * _KW, _C, _CO], fp32, kind="ExternalInput")
    out = nc.dram_tensor("out", [npc, _CO, _HO, _WO], fp32, kind="ExternalOutput")

    n_ci = _C // 128   # 2
    n_co = _CO // 128  # 2
    nk = _KH * _KW     # 9
    WP = _WO // 2      # 27 output pairs
    G = _HO // rg      # row groups (rg=18 -> 3)
    assert _HO % rg == 0

    with tile.TileContext(nc) as tc:
        with (
            tc.tile_pool(name="wstage", bufs=1) as wstage_pool,
            tc.tile_pool(name="wsign", bufs=1) as wsign_pool,
            tc.tile_pool(name="wtmp", bufs=2) as wtmp_pool,
            tc.tile_pool(name="wg", bufs=n_ci) as wg_pool,
            tc.tile_pool(name="xf", bufs=2) as xf_pool,
            tc.tile_pool(name="xw", bufs=xw_bufs) as xw_pool,
            tc.tile_pool(name="dt", bufs=dt_bufs) as dt_pool,
            tc.tile_pool(name="ob", bufs=ob_bufs) as ob_pool,
            tc.tile_pool(name="ps", bufs=8, space="PSUM") as ps_pool,
        ):
            # ---- weight prep: binarize then 1D-transform along kw.
            # wg[ci_t][:, kh*4 + c, co]: c=0 -> s0, c=1 -> (s0+s1+s2)/2,
            # c=2 -> (s0-s1+s2)/2, c=3 -> s2   (s = sign(w))
            wg = {}
            for ci_t in range(n_ci):
                stage = wstage_pool.tile([128, nk, _CO], fp32)
                nc.scalar.dma_start(
                    stage[:],
                    w_in[:, ci_t * 128 : (ci_t + 1) * 128, :].rearrange(
                        "k c o -> c k o"
                    ),
                )
                sg = wsign_pool.tile([128, nk, _CO], fp32)
                nc.vector.tensor_scalar(
                    sg[:], stage[:], 0.0, 0.5,
                    mybir.AluOpType.is_ge, mybir.AluOpType.subtract,
                )  # +-0.5 = sign/2
                wt = wg_pool.tile([128, _KH * 4, _CO], bf16)
                for kh in range(_KH):
                    s0 = sg[:, kh * _KW + 0, :]
                    s1 = sg[:, kh * _KW + 1, :]
                    s2 = sg[:, kh * _KW + 2, :]
                    # c=0: s0 (x2 undoes the half encoding)
                    nc.scalar.mul(wt[:, kh * 4 + 0, :], s0, 2.0)
                    # c=3: s2
                    nc.scalar.mul(wt[:, kh * 4 + 3, :], s2, 2.0)
                    t02 = wtmp_pool.tile([128, _CO], fp32, name="t02", tag="t02")
                    nc.vector.tensor_add(t02[:], s0, s2)
                    tp = wtmp_pool.tile([128, _CO], fp32, name="tp", tag="tp")
                    nc.vector.tensor_add(tp[:], t02[:], s1)
                    nc.scalar.copy(wt[:, kh * 4 + 1, :], tp[:])  # (s0+s1+s2)/2
                    tm = wtmp_pool.tile([128, _CO], fp32, name="tm", tag="tm")
                    nc.vector.tensor_sub(tm[:], t02[:], s1)
                    nc.scalar.copy(wt[:, kh * 4 + 2, :], tm[:])  # (s0-s1+s2)/2
                wg[ci_t] = wt

            loop_cm = (
                tc.For_i(
                    0,
                    hw_loop,
                    1,
                    hint_engines=(
                        mybir.EngineType.PE,
                        mybir.EngineType.Activation,
                        mybir.EngineType.DVE,
                        mybir.EngineType.SP,
                    ),
                )
                if hw_loop > 0
                else contextlib.nullcontext()
            )
            with loop_cm:
                for rep in range(reps):
                    for n in range(npc):
                        # input comps straight from f32 (cast fused into sub/add)
                        xw = {}
                        for ci_t in range(n_ci):
                            xf = xf_pool.tile([128, _H * _W], fp32)
                            nc.sync.dma_start(
                                xf[:], x_in[n, ci_t * 128 : (ci_t + 1) * 128, :, :]
                            )
                            v = xf[:].rearrange(
                                "p (h wp t) -> p h wp t", wp=_W // 2, t=2
                            )
                            d0 = v[:, :, 0:WP, 0]
                            d1 = v[:, :, 0:WP, 1]
                            d2 = v[:, :, 1 : WP + 1, 0]
                            d3 = v[:, :, 1 : WP + 1, 1]
                            xc = xw_pool.tile(
                                [128, 4, _H, WP], bf16, name="xc", tag="xc"
                            )
                            # chunk over rows so group-0 matmuls start before
                            # the whole image is transformed
                            hstep = -(-_H // tr_chunks)
                            for h0 in range(0, _H, hstep):
                                h1 = min(h0 + hstep, _H)
                                r = slice(h0, h1)
                                nc.vector.tensor_sub(
                                    xc[:, 0, r], d0[:, r], d2[:, r]
                                )  # c1
                                nc.vector.tensor_add(
                                    xc[:, 1, r], d1[:, r], d2[:, r]
                                )  # c2
                                nc.vector.tensor_sub(
                                    xc[:, 2, r], d2[:, r], d1[:, r]
                                )  # c3
                                nc.vector.tensor_sub(
                                    xc[:, 3, r], d1[:, r], d3[:, r]
                                )  # c4
                            xw[ci_t] = xc

                        for co_t in range(n_co):
                            ob = ob_pool.tile(
                                [128, _HO, _WO], fp32, name="ob", tag="ob"
                            )
                            obv = ob[:].rearrange("p h (wp t) -> p h wp t", t=2)
                            for g in range(G):
                                mq = [
                                    ps_pool.tile(
                                        [128, rg, WP], fp32, name="mq", tag="mq"
                                    )
                                    for _ in range(4)
                                ]
                                for c in range(4):
                                    for ci_t in range(n_ci):
                                        for kh in range(_KH):
                                            # comp c uses weight col c of tap kh
                                            w_ap = wg[ci_t][
                                                :, kh * 4 + c,
                                                co_t * 128 : (co_t + 1) * 128,
                                            ]
                                            rhs = xw[ci_t][
                                                :, c, g * rg + kh : g * rg + kh + rg, :
                                            ]
                                            nc.tensor.matmul(
                                                mq[c][:, :, :],
                                                w_ap,
                                                rhs,
                                                start=(ci_t == 0 and kh == 0),
                                                stop=(
                                                    ci_t == n_ci - 1 and kh == _KH - 1
                                                ),
                                            )
                                # detransform: y0=m1+m2+m3, y1=m2-m3-m4.
                                # DVE may read only ONE psum operand per op;
                                # ACT stages m2,m3 into SBUF first.
                                rows = slice(g * rg, (g + 1) * rg)
                                s2 = dt_pool.tile([128, rg, WP], fp32, name="s2", tag="s2")
                                nc.scalar.copy(s2[:], mq[1][:])
                                s3 = dt_pool.tile([128, rg, WP], fp32, name="s3", tag="s3")
                                nc.scalar.copy(s3[:], mq[2][:])
                                t0 = dt_pool.tile([128, rg, WP], fp32, name="t0", tag="t0")
                                nc.vector.tensor_add(t0[:], mq[0][:], s2[:])
                                nc.vector.tensor_add(
                                    obv[:, rows, :, 0], t0[:], s3[:]
                                )
                                t1 = dt_pool.tile([128, rg, WP], fp32, name="t1", tag="t1")
                                nc.vector.tensor_sub(t1[:], s2[:], s3[:])
                                nc.vector.tensor_sub(
                                    obv[:, rows, :, 1], t1[:], mq[3][:]
                                )
                                if g == G - 1:
                                    nc.scalar.dma_start(
                                        out[
                                            n,
                                            co_t * 128 : (co_t + 1) * 128,
                                            :,
                                            :,
                                        ],
                                        ob[:],
                                    )

    nc.compile()
    return nc


def build_kernel(
    npc=_NPC,
    reps=1,
    hw_loop=0,
    dma_chunks=2,
    xf_bufs=6,
    xb_bufs=4,
    xw_bufs=4,
    dt_bufs=3,
    ob_bufs=4,
    ps_bufs=8,
    cast_dve=False,
    out_whole=False,
    flat_rhs=True,
    dt_bf16=False,
):
    """1D Winograd F(2,3) along HEIGHT (v2).

    vs build_conv_bass_wino (width-direction):
      - all input-transform reads are unit-stride rows -> bf16
        tensor_tensor runs in the DVE 2x packed mode (width version's
        stride-2 column slices force 1x);
      - the f32->bf16 cast runs on ACT (activation Copy), freeing DVE;
      - input DMA is row-chunked and ci-interleaved so the first matmuls
        of each image start earlier;
      - output DMA goes out per row-group (3 per co-tile) so the
        loop-tail after the last matmul is short.

    Math per output row pair (y0,y1) at rows (2i, 2i+1), per (ci, kw):
      r_t = x[2i+t, :]  (t=0..3)
      c1 = r0-r2, c2 = r1+r2, c3 = r2-r1, c4 = r1-r3      (bf16, exact +-)
      m_k = sum_{ci,kw} c_k * wk[ci,kw]  with
      w1 = s[kh=0], w2 = (s0+s1+s2)/2, w3 = (s0-s1+s2)/2, w4 = s[kh=2]
      y0 = m1+m2+m3,  y1 = m2-m3-m4
    s = sign(W) so all transformed weights are in {+-1, +-1/2, +-3/2} --
    exact in bf16.  kw taps are handled directly via rhs column shifts.
    """
    import contextlib

    import concourse.mybir as mybir
    import concourse.tile as tile
    from concourse import bacc

    fp32 = mybir.dt.float32
    bf16 = mybir.dt.bfloat16

    nc = bacc.Bacc("TRN2", target_bir_lowering=False, debug=False)

    x_in = nc.dram_tensor("x", [npc, _C, _H, _W], fp32, kind="ExternalInput")
    w_in = nc.dram_tensor("w", [_KH * _KW, _C, _CO], fp32, kind="ExternalInput")
    out = nc.dram_tensor("out", [npc, _CO, _HO, _WO], fp32, kind="ExternalOutput")

    n_ci = _C // 128   # 2
    n_co = _CO // 128  # 2
    nk = _KH * _KW     # 9
    HP = _HO // 2      # 27 output row pairs
    RG = 9             # pairs per PSUM group ([128, 9, 54] f32 = 1944B = 1 bank)
    G = HP // RG       # 3 row-pair groups
    assert _H % (2 * dma_chunks) == 0
    hch = _H // dma_chunks  # rows per DMA chunk (even)

    with tile.TileContext(nc) as tc:
        with (
            tc.tile_pool(name="wstage", bufs=1) as wstage_pool,
            tc.tile_pool(name="wsign", bufs=1) as wsign_pool,
            tc.tile_pool(name="wtmp", bufs=2) as wtmp_pool,
            tc.tile_pool(name="wg", bufs=n_ci) as wg_pool,
            tc.tile_pool(name="xf", bufs=xf_bufs) as xf_pool,
            tc.tile_pool(name="xb", bufs=xb_bufs) as xb_pool,
            tc.tile_pool(name="xw", bufs=xw_bufs) as xw_pool,
            tc.tile_pool(name="dt", bufs=dt_bufs) as dt_pool,
            tc.tile_pool(name="ob", bufs=ob_bufs) as ob_pool,
            tc.tile_pool(name="ps", bufs=ps_bufs, space="PSUM") as ps_pool,
        ):
            # ---- weight prep (outside the rep loop): binarize, then the
            # height transform over kh.  wg[ci_t][:, c*3 + kw, co]:
            #   c=0 -> s[kh0], c=1 -> (s0+s1+s2)/2, c=2 -> (s0-s1+s2)/2,
            #   c=3 -> s[kh2]          (s = sign(w), stored directly in bf16)
            wg = {}
            for ci_t in range(n_ci):
                stage = wstage_pool.tile([128, nk, _CO], fp32)
                nc.scalar.dma_start(
                    stage[:],
                    w_in[:, ci_t * 128 : (ci_t + 1) * 128, :].rearrange(
                        "k c o -> c k o"
                    ),
                )
                sg = wsign_pool.tile([128, nk, _CO], fp32)
                nc.vector.tensor_scalar(
                    sg[:], stage[:], 0.0, 0.5,
                    mybir.AluOpType.is_ge, mybir.AluOpType.subtract,
                )  # +-0.5 = sign/2
                wt = wg_pool.tile([128, 4 * _KW, _CO], bf16)
                for kw in range(_KW):
                    s0 = sg[:, 0 * _KW + kw, :]
                    s1 = sg[:, 1 * _KW + kw, :]
                    s2 = sg[:, 2 * _KW + kw, :]
                    nc.scalar.mul(wt[:, 0 * _KW + kw, :], s0, 2.0)  # c=0
                    nc.scalar.mul(wt[:, 3 * _KW + kw, :], s2, 2.0)  # c=3
                    t02 = wtmp_pool.tile([128, _CO], fp32, name="t02", tag="t02")
                    nc.vector.tensor_add(t02[:], s0, s2)
                    tp = wtmp_pool.tile([128, _CO], fp32, name="tp", tag="tp")
                    nc.vector.tensor_add(tp[:], t02[:], s1)
                    nc.scalar.copy(wt[:, 1 * _KW + kw, :], tp[:])  # (s0+s1+s2)/2
                    tm = wtmp_pool.tile([128, _CO], fp32, name="tm", tag="tm")
                    nc.vector.tensor_sub(tm[:], t02[:], s1)
                    nc.scalar.copy(wt[:, 2 * _KW + kw, :], tm[:])  # (s0-s1+s2)/2
                wg[ci_t] = wt

            loop_cm = (
                tc.For_i(
                    0,
                    hw_loop,
                    1,
                    hint_engines=(
                        mybir.EngineType.PE,
                        mybir.EngineType.Activation,
                        mybir.EngineType.DVE,
                        mybir.EngineType.SP,
                    ),
                )
                if hw_loop > 0
                else contextlib.nullcontext()
            )
            with loop_cm:
                for rep in range(reps):
                    for n in range(npc):
                        # ---- input: chunked DMA (ci-interleaved), ACT cast
                        # to bf16, DVE height-transform (all unit-stride).
                        xb = {}
                        for ci_t in range(n_ci):
                            xb[ci_t] = xb_pool.tile(
                                [128, _H, _W], bf16, name="xb", tag="xb"
                            )
                        for ch in range(dma_chunks):
                            r0, r1 = ch * hch, (ch + 1) * hch
                            for ci_t in range(n_ci):
                                xf = xf_pool.tile([128, hch, _W], fp32)
                                nc.sync.dma_start(
                                    xf[:],
                                    x_in[n, ci_t * 128 : (ci_t + 1) * 128, r0:r1, :],
                                )
                                if cast_dve:
                                    nc.vector.tensor_copy(
                                        xb[ci_t][:, r0:r1, :], xf[:]
                                    )
                                else:
                                    nc.scalar.copy(xb[ci_t][:, r0:r1, :], xf[:])
                        xw = {}
                        xwf = {}
                        for ci_t in range(n_ci):
                            if flat_rhs:
                                # padded flat tile so the last flat matmul
                                # window (base kw=2) stays in bounds
                                xcf = xw_pool.tile(
                                    [128, 4 * HP * _W + 8], bf16,
                                    name="xc", tag="xc",
                                )
                                nc.vector.memset(xcf[:, 4 * HP * _W :], 0.0)
                                xc = xcf[:, : 4 * HP * _W].rearrange(
                                    "p (c i w) -> p c i w", c=4, i=HP
                                )
                                xwf[ci_t] = xcf
                            else:
                                xc = xw_pool.tile(
                                    [128, 4, HP, _W], bf16, name="xc", tag="xc"
                                )
                            # v[:, i, t, :] = input row 2i+t
                            v = xb[ci_t][:].rearrange(
                                "p (i t) w -> p i t w", t=2
                            )
                            # chunk the transform so group-0 matmuls can
                            # start before the whole image is cast
                            for ch in range(dma_chunks):
                                # pairs fully determined by rows < (ch+1)*hch
                                p1 = min(((ch + 1) * hch - 2) // 2, HP)
                                p0 = max((ch * hch - 2) // 2, 0) if ch else 0
                                if p1 <= p0:
                                    continue
                                pr = slice(p0, p1)
                                r0 = v[:, p0:p1, 0, :]
                                r1s = v[:, p0:p1, 1, :]
                                r2 = v[:, p0 + 1 : p1 + 1, 0, :]
                                r3 = v[:, p0 + 1 : p1 + 1, 1, :]
                                nc.vector.tensor_sub(xc[:, 0, pr, :], r0, r2)
                                nc.vector.tensor_add(xc[:, 1, pr, :], r1s, r2)
                                nc.vector.tensor_sub(xc[:, 2, pr, :], r2, r1s)
                                nc.vector.tensor_sub(xc[:, 3, pr, :], r1s, r3)
                            xw[ci_t] = xc

                        for co_t in range(n_co):
                            if out_whole:
                                obw = ob_pool.tile(
                                    [128, _HO, _WO], fp32, name="obw", tag="obw"
                                )
                            for g in range(G):
                                pshape = (
                                    [128, RG * _W] if flat_rhs else [128, RG, _WO]
                                )
                                mq = [
                                    ps_pool.tile(
                                        pshape, fp32, name="mq", tag="mq"
                                    )
                                    for _ in range(4)
                                ]
                                for c in range(4):
                                    for ci_t in range(n_ci):
                                        for kw in range(_KW):
                                            w_ap = wg[ci_t][
                                                :, c * _KW + kw,
                                                co_t * 128 : (co_t + 1) * 128,
                                            ]
                                            if flat_rhs:
                                                base = (
                                                    (c * HP + g * RG) * _W + kw
                                                )
                                                rhs = xwf[ci_t][
                                                    :, base : base + RG * _W
                                                ]
                                            else:
                                                rhs = xw[ci_t][
                                                    :, c,
                                                    g * RG : (g + 1) * RG,
                                                    kw : kw + _WO,
                                                ]
                                            nc.tensor.matmul(
                                                mq[c][:],
                                                w_ap,
                                                rhs,
                                                start=(ci_t == 0 and kw == 0),
                                                stop=(
                                                    ci_t == n_ci - 1
                                                    and kw == _KW - 1
                                                ),
                                            )
                                if flat_rhs:
                                    mv = [
                                        m[:].rearrange(
                                            "p (r w) -> p r w", w=_W
                                        )[:, :, 0:_WO]
                                        for m in mq
                                    ]
                                else:
                                    mv = [m[:] for m in mq]
                                # detransform: y0 = m1+m2+m3, y1 = m2-m3-m4.
                                # DVE reads only one PSUM operand per op;
                                # ACT stages m3 into SBUF.
                                if out_whole:
                                    ob = obw
                                    obv = ob[:].rearrange(
                                        "p (i t) w -> p i t w", t=2
                                    )[:, g * RG : (g + 1) * RG]
                                else:
                                    ob = ob_pool.tile(
                                        [128, 2 * RG, _WO], fp32,
                                        name="ob", tag="ob",
                                    )
                                    obv = ob[:].rearrange(
                                        "p (i t) w -> p i t w", t=2
                                    )
                                if dt_bf16:
                                    # stage m2, m3 to bf16 so p/q run in the
                                    # DVE 2x packed mode; the ~0.1% extra
                                    # rounding is far inside the tolerance
                                    a2 = dt_pool.tile(
                                        [128, RG, _WO], bf16, name="a2", tag="a2"
                                    )
                                    nc.scalar.copy(a2[:], mv[1])
                                    a3 = dt_pool.tile(
                                        [128, RG, _WO], bf16, name="a3", tag="a3"
                                    )
                                    nc.scalar.copy(a3[:], mv[2])
                                    p = dt_pool.tile(
                                        [128, RG, _WO], bf16, name="p", tag="p"
                                    )
                                    nc.vector.tensor_add(p[:], a2[:], a3[:])
                                    q = dt_pool.tile(
                                        [128, RG, _WO], bf16, name="q", tag="q"
                                    )
                                    nc.vector.tensor_sub(q[:], a2[:], a3[:])
                                else:
                                    a3 = dt_pool.tile(
                                        [128, RG, _WO], fp32, name="a3", tag="a3"
                                    )
                                    nc.scalar.copy(a3[:], mv[2])
                                    p = dt_pool.tile(
                                        [128, RG, _WO], fp32, name="p", tag="p"
                                    )
                                    nc.vector.tensor_add(p[:], mv[1], a3[:])
                                    q = dt_pool.tile(
                                        [128, RG, _WO], fp32, name="q", tag="q"
                                    )
                                    nc.vector.tensor_sub(q[:], mv[1], a3[:])
                                nc.vector.tensor_add(
                                    obv[:, :, 0, :], p[:], mv[0]
                                )
                                nc.vector.tensor_sub(
                                    obv[:, :, 1, :], q[:], mv[3]
                                )
                                if not out_whole:
                                    nc.scalar.dma_start(
                                        out[
                                            n,
                                            co_t * 128 : (co_t + 1) * 128,
                                            2 * RG * g : 2 * RG * (g + 1),
                                            :,
                                        ],
                                        ob[:],
                                    )
                                elif g == G - 1:
                                    nc.scalar.dma_start(
                                        out[
                                            n,
                                            co_t * 128 : (co_t + 1) * 128,
                                            :,
                                            :,
                                        ],
                                        obw[:],
                                    )

    nc.compile()
    return nc


_CACHED_NC = None


def _host_weight_layout(W):
    # OIHW [co,ci,kh,kw] -> [kh*kw, ci, co]; layout only, no arithmetic.
    return np.ascontiguousarray(
        np.transpose(np.asarray(W, dtype=np.float32), (2, 3, 1, 0)).reshape(
            _KH * _KW, _C, _CO
        )
    )


def kernel(X, W):
    import os

    # NTFF tracing is unavailable under this axon image (antenv.axon_hooks
    # missing); make sure a stray BASS_TRACE can't route us into it.
    os.environ["BASS_NEVER_TRACE"] = "1"
    from concourse.bass_utils import run_bass_kernel_spmd

    global _CACHED_NC
    if _CACHED_NC is None:
        _CACHED_NC = build_kernel(_NPC)
    nc = _CACHED_NC

    X = np.asarray(X, dtype=np.float32)
    Wt = _host_weight_layout(W)

    in_maps = [
        {"x": X[c * _NPC : (c + 1) * _NPC], "w": Wt} for c in range(_NCORES)
    ]
    res = run_bass_kernel_spmd(nc, in_maps, core_ids=list(range(_NCORES)))
    return np.concatenate([res.results[c]["out"] for c in range(_NCORES)], axis=0)

